# revision 44
# baseline (speedup 1.0000x reference)
"""BatchAllTripletLoss on 8 Trainium2 NeuronCores via Bass/Tile.

Math: for anchors i, positives j (same label, j!=i), negatives k (diff label):
  total        = sum_{i,j,k} relu(d_ij - d_ik + margin)
  num_non_easy = #{(i,j,k): d_ik < d_ij + margin}
  loss         = total / num_non_easy ; frac = num_non_easy / num_valid

Strategy. Anchors are grouped into UNITS of up to two anchors from two
different labels (side A at partition 0, side B at partition SPLIT).
Host-built select-masks compact each unit's positive thresholds
t = d_ij + margin onto the 128 partitions; the unit's compare input is
v' = sqrt(d2 + BIG*same_label) for both anchors, broadcast to [128, 640].

Per unit, two reduction passes produce everything via engine accumulators
(accum_out), read back as per-unit columns and summed on host:
  count row sums: R += sum_k (v' < t)        [is_lt, accum]
  relu row sums:  W += sum_k min(v'-t, 0)    [= -relu sums; DVE/Pool]
               or W += sum_k relu(t-v')      [ACT Relu, scale=-1, bias=t]

The v' broadcast is fed two ways to balance engines:
  D-units: DRAM-staged broadcast DMA (partition-stride-0 src) into bf16
    group tiles; compare+relu on DVE (2x bf16 mode) / Pool / ACT.
  P-units: PE broadcast matmul (constant selector weights, staged 2-row
    rhs at partitions 0-1) into PSUM f32; Pool compares, ACT relu-sums.

Distances: one bf16 PE matmul chain per core (dot + rank-1 norm updates);
norms, the BIG same-label mask, and all bf16 casts are host-precomputed.
num_valid is pure label counting (host, exact).
"""

import os
import numpy as np
import ml_dtypes

BF16 = ml_dtypes.bfloat16

# bisection switches (crash isolation); default off
DBG_NO_TSUM = bool(int(os.environ.get("DBG_NO_TSUM", "0")))
DBG_NO_P = bool(int(os.environ.get("DBG_NO_P", "0")))
DBG_ALL_ACT = bool(int(os.environ.get("DBG_ALL_ACT", "0")))
DBG_NO_Q = bool(int(os.environ.get("DBG_NO_Q", "0")))

N = 640
D = 128
NCORES = 8
UNITS = 60                    # units per core (re-derived from labels)
ALOC = 2 * UNITS              # anchor slots per core (2 per unit)
NBLK = 3                      # k-tiles that may hold label blocks
SPLIT = 64                    # partition row where side-B blocks start
MARGIN = 1.9
BIG = 1.0e9
PREFETCH = 2                  # vb group prefetch depth
ND = 40                       # D-units (DMA-fed); rest are P-units (PE-fed)


def _vb_groups():
    """Ramped vb group sizes summing to ND: early compares start after a
    small first transfer instead of a multi-MB one."""
    sizes = []
    for s in (2, 2, 4):
        if sum(sizes) + s <= ND:
            sizes.append(s)
    while sum(sizes) < ND:
        sizes.append(min(8, ND - sum(sizes)))
    return sizes

_CACHE = {}

# D-unit relu engine pattern (cycle of 8): 5 DVE, 3 ACT. (Pool cannot
# run accumulating ops; it serves the P-unit count masks instead.)
_RELU_PAT = ("D", "D", "A", "D", "D", "A", "D", "A")


def _relu_engine(u):
    """Engine for the relu-sum pass of unit u. P-units always ACT."""
    if u >= ND or DBG_ALL_ACT:
        return "A"
    return _RELU_PAT[u % 8]


def _build_program():
    import concourse.bass as bass
    import concourse.bacc as bacc
    import concourse.mybir as mybir
    import concourse.tile as tile
    from concourse.masks import make_identity

    f32 = mybir.dt.float32
    bf16 = mybir.dt.bfloat16
    Alu = mybir.AluOpType
    Act = mybir.ActivationFunctionType

    NP = UNITS - ND           # P-units

    nc = bacc.Bacc("TRN2", target_bir_lowering=False, debug=False,
                   num_devices=NCORES)

    # efT columns pre-scaled by -2 on host; all bf16 host-precast.
    efT2 = nc.declare_dram_parameter("efT2", [D, N], bf16, isOutput=False)
    elocT = nc.declare_dram_parameter("elocT", [D, ALOC], bf16, isOutput=False)
    sqfb = nc.declare_dram_parameter("sqfb", [1, N], bf16, isOutput=False)
    sqlb = nc.declare_dram_parameter("sqlb", [1, ALOC], bf16, isOutput=False)
    eqb = nc.declare_dram_parameter("eqb", [ALOC, N], bf16, isOutput=False)
    selm = nc.declare_dram_parameter("selm", [128, NBLK * 2 * UNITS], bf16,
                                     isOutput=False)
    selpat = nc.declare_dram_parameter("selpat", [2, 128], bf16,
                                       isOutput=False)
    rd_o = nc.declare_dram_parameter("rd", [128, UNITS], f32, isOutput=True)
    wd_o = nc.declare_dram_parameter("wd", [128, UNITS], f32, isOutput=True)
    wa_o = nc.declare_dram_parameter("wa", [128, UNITS], f32, isOutput=True)
    qc_o = nc.declare_dram_parameter("qc", [1, N], f32, isOutput=True)
    tsum_o = nc.declare_dram_parameter("tsum", [1, UNITS], f32, isOutput=True)

    with tile.TileContext(nc) as tc:
        with (
            tc.tile_pool(name="singles", bufs=1) as sg,
            tc.tile_pool(name="vbp", bufs=PREFETCH + 1) as vbp,
            tc.tile_pool(name="jD", bufs=4) as jDp,
            tc.tile_pool(name="jP", bufs=3) as jPp,
            tc.tile_pool(name="jA", bufs=3) as jAp,
            tc.tile_pool(name="dram", bufs=1, space="DRAM") as dram,
            tc.tile_pool(name="ps_a", bufs=1, space="PSUM") as ps_a,
            tc.tile_pool(name="ps_pb", bufs=3, space="PSUM") as ps_pb,
        ):
            # ---- load inputs ----
            EF2 = sg.tile([D, N], bf16)
            nc.sync.dma_start(out=EF2[:], in_=efT2[:])
            EL = sg.tile([D, ALOC], bf16)
            nc.scalar.dma_start(out=EL[:], in_=elocT[:])
            SQFB = sg.tile([1, N], bf16)
            nc.scalar.dma_start(out=SQFB[:], in_=sqfb[:])
            SQLB = sg.tile([1, ALOC], bf16)
            nc.scalar.dma_start(out=SQLB[:], in_=sqlb[:])
            SELPB = sg.tile([2, 128], bf16)
            nc.scalar.dma_start(out=SELPB[:], in_=selpat[:])
            EQBB = sg.tile([ALOC, N], bf16)
            nc.sync.dma_start(out=EQBB[:], in_=eqb[:])
            SELM = sg.tile([128, NBLK * 2 * UNITS], bf16)
            nc.gpsimd.dma_start(out=SELM[:], in_=selm[:])

            ident = sg.tile([128, 128], f32)
            make_identity(nc, ident[:])
            ONESB = sg.tile([1, N], bf16)
            nc.vector.memset(ONESB[:], 1.0)
            ONESC = sg.tile([128, 1], bf16)
            nc.vector.memset(ONESC[:], 1.0)
            ONESCF = sg.tile([128, 1], f32)
            nc.vector.memset(ONESCF[:], 1.0)

            # accumulator result tiles (zeroed: dummy cols stay 0)
            RD = sg.tile([128, UNITS], f32)
            nc.vector.memset(RD[:], 0.0)
            WD = sg.tile([128, UNITS], f32)
            nc.vector.memset(WD[:], 0.0)
            WA = sg.tile([128, UNITS], f32)
            nc.vector.memset(WA[:], 0.0)

            # ---- squared distances fully in PSUM (bf16 matmuls) ----
            # d2[a,k] = (-2 e_a).e_k + |e_k|^2 + |e_a|^2
            dot_ps = ps_a.tile([ALOC, N], f32, tag="a", name="dot")
            for c0, c1 in ((0, 512), (512, N)):
                nc.tensor.matmul(dot_ps[:, c0:c1], EL[:], EF2[:, c0:c1],
                                 start=True, stop=False)
                nc.tensor.matmul(dot_ps[:, c0:c1], ONESB[:, 0:ALOC],
                                 SQFB[:, c0:c1], start=False, stop=False)
                nc.tensor.matmul(dot_ps[:, c0:c1], SQLB[:],
                                 ONESB[:, 0:c1 - c0], start=False, stop=True)

            # PRE2 = max(dot,0); PREM = PRE2 + BIG*eq; VMB = sqrt(PREM) bf16
            PRE2 = sg.tile([ALOC, N], f32)
            nc.vector.tensor_scalar(out=PRE2[:], in0=dot_ps[:], scalar1=0.0,
                                    scalar2=None, op0=Alu.max)
            PREM = sg.tile([ALOC, N], f32)
            nc.vector.scalar_tensor_tensor(out=PREM[:], in0=dot_ps[:],
                                           scalar=0.0, in1=EQBB[:],
                                           op0=Alu.max, op1=Alu.add)
            VMB = sg.tile([ALOC, N], bf16)
            nc.scalar.activation(out=VMB[:], in_=PREM[:], func=Act.Sqrt)
            DIST = sg.tile([ALOC, N], f32)
            nc.scalar.activation(out=DIST[:], in_=PRE2[:], func=Act.Sqrt)

            # ---- P-unit staging: rows (A, B) of each P-unit at parts 0/1
            VMBP = sg.tile([2, NP * N], bf16)
            nc.gpsimd.dma_start(out=VMBP[0:1, :], in_=VMB[ND:UNITS, :])
            nc.gpsimd.dma_start(out=VMBP[1:2, :],
                                in_=VMB[UNITS + ND:2 * UNITS, :])

            # ---- D-unit staging: VMB rows for D-units to DRAM (broadcast
            # DMA requires a DRAM source; SBUF APs reject partition step 0)
            # NOTE: the staging write must be on a DIFFERENT queue than the
            # vb broadcast reads below (sync): same-queue DMAs issue FIFO
            # but complete out of order across the 16 engines, so a reader
            # on the same queue can see stale DRAM.
            vmd = dram.tile([2, ND, N], bf16)
            nc.scalar.dma_start(out=vmd[0:1, :, :], in_=VMB[0:ND, :])
            nc.scalar.dma_start(out=vmd[1:2, :, :],
                                in_=VMB[UNITS:UNITS + ND, :])

            # ---- vb group DMA for D-units (DRAM->SBUF broadcast) ----
            # Triggers go on the otherwise-idle SP (sync) queue; ramped
            # group sizes so the first compares start after a small
            # transfer instead of a multi-MB one.
            VB_GROUPS = _vb_groups()
            gstart = []
            s = 0
            for gsz in VB_GROUPS:
                gstart.append(s)
                s += gsz
            assert s == ND, (s, ND)
            MAXG = max(VB_GROUPS)
            vb_tiles = {}

            def issue_vb(g):
                if g >= len(VB_GROUPS):
                    return
                u0 = gstart[g]
                nu = VB_GROUPS[g]
                vb8 = vbp.tile([128, MAXG, N], bf16, tag="vb", name="vb")
                pitch = MAXG * N
                for half, (p0, np_) in enumerate(((0, SPLIT),
                                                  (SPLIT, 128 - SPLIT))):
                    dst = bass.AP(tensor=vb8.tensor,
                                  offset=vb8.offset + p0 * pitch,
                                  ap=[[pitch, np_], [N, nu], [1, N]])
                    src = bass.AP(tensor=vmd.tensor,
                                  offset=vmd.offset + (half * ND + u0) * N,
                                  ap=[[0, np_], [N, nu], [1, N]])
                    nc.sync.dma_start(out=dst, in_=src)
                vb_tiles[g] = vb8

            u2g = {}
            for g, (u0, gsz) in enumerate(zip(gstart, VB_GROUPS)):
                for i in range(gsz):
                    u2g[u0 + i] = g

            for g in range(PREFETCH):
                issue_vb(g)

            # ---- thresholds: TSEL[p,u] from transposed DIST + masks ----
            TSEL = sg.tile([128, UNITS], f32)
            first = True
            for c in range(NBLK):
                tr_ps = ps_a.tile([128, ALOC], f32, tag="a", name=f"tr{c}")
                nc.tensor.transpose(tr_ps[:], DIST[:, c * 128:(c + 1) * 128],
                                    ident[0:ALOC, 0:ALOC])
                for side in range(2):
                    sel = SELM[:, (2 * c + side) * UNITS:
                               (2 * c + side + 1) * UNITS]
                    if first:
                        nc.vector.scalar_tensor_tensor(
                            out=TSEL[:],
                            in0=tr_ps[:, side * UNITS:(side + 1) * UNITS],
                            scalar=MARGIN, in1=sel,
                            op0=Alu.add, op1=Alu.mult)
                        first = False
                    else:
                        t = sg.tile([128, UNITS], f32, tag="tstmp",
                                    name="tstmp")
                        nc.vector.scalar_tensor_tensor(
                            out=t[:],
                            in0=tr_ps[:, side * UNITS:(side + 1) * UNITS],
                            scalar=MARGIN, in1=sel,
                            op0=Alu.add, op1=Alu.mult)
                        nc.vector.tensor_add(TSEL[:], TSEL[:], t[:])

            # ---- main loop: riffle P and D units (P first: their staging
            # DMA is ready before the D-units' DRAM round-trip) ----
            order = []
            di, pi = 0, ND
            while di < ND or pi < UNITS:
                take_p = (pi - ND) * ND <= di * (UNITS - ND)
                if pi < UNITS and (take_p or di >= ND):
                    order.append(pi); pi += 1
                else:
                    order.append(di); di += 1

            # per-unit threshold column sums (exact, device-side): the DVE
            # relu pass accumulates sum_k min(vb, t) and the host recovers
            # sum relu(t-vb) = 640*tsum - that, so tsum must match the
            # device's own t values bit-for-bit (no host sqrt recompute).
            TSUM = sg.tile([1, UNITS], f32)
            if not DBG_NO_TSUM:
                ts_ps = ps_a.tile([1, UNITS], f32, tag="a", name="tsum")
                nc.tensor.matmul(ts_ps[:], ONESCF[:], TSEL[:])
                nc.vector.tensor_copy(TSUM[:], ts_ps[:])
            else:
                nc.vector.memset(TSUM[:], 0.0)

            # P-unit count accumulator: reuses the startup psum slot (its
            # last prior tile, ts_ps, is fully consumed by the copy)
            q1 = None
            if not (DBG_NO_P or DBG_NO_Q):
                q1 = ps_a.tile([1, N], f32, tag="a", name="q1")
            pend_mq = []          # masks awaiting a PE colsum (LDW batching)
            np_seen = [0]
            NPU = UNITS - ND

            def flush_colsums(final=False):
                # Each flush is its own short accumulation group (stop at
                # flush end, reopened with start=False): a single group held
                # open across the whole loop with interleaved broadcast
                # matmuls wedges the PE.
                for i, (mq, is_first) in enumerate(pend_mq):
                    last = i == len(pend_mq) - 1
                    nc.tensor.matmul(q1[:, 0:512], ONESC[:], mq[:, 0:512],
                                     start=is_first, stop=last,
                                     skip_group_check=True)
                    nc.tensor.matmul(q1[:, 512:N], ONESC[:], mq[:, 512:N],
                                     start=is_first, stop=last,
                                     skip_group_check=True)
                pend_mq.clear()

            for u in order:
                if u >= ND and DBG_NO_P:
                    continue
                tcol = TSEL[:, u:u + 1]
                if u < ND:
                    g = u2g[u]
                    if u == gstart[g]:
                        issue_vb(g + PREFETCH)
                    vb = vb_tiles[g][:, u - gstart[g], :]
                    md = jDp.tile([128, N], bf16, tag="jD", name="jD")
                    nc.vector.tensor_scalar(out=md[:], in0=vb, scalar1=tcol,
                                            scalar2=0.0, op0=Alu.is_lt,
                                            op1=Alu.add,
                                            accum_out=RD[:, u:u + 1])
                    if _relu_engine(u) == "D":
                        # accum (op1) is the REDUCE op; op0=min gives
                        # sum_k min(vb, t) = 640*t - sum_k relu(t - vb)
                        md2 = jDp.tile([128, N], bf16, tag="jD", name="jD2")
                        nc.vector.tensor_scalar(out=md2[:], in0=vb,
                                                scalar1=tcol, scalar2=0.0,
                                                op0=Alu.min, op1=Alu.add,
                                                accum_out=WD[:, u:u + 1])
                    else:
                        ja2 = jAp.tile([128, N], bf16, tag="jA", name="jA2")
                        nc.scalar.activation(out=ja2[:], in_=vb,
                                             func=Act.Relu, bias=tcol,
                                             scale=-1.0,
                                             accum_out=WA[:, u:u + 1])
                else:
                    pi_ = u - ND
                    pb = ps_pb.tile([128, N], f32, tag="pb", name="pb")
                    nc.tensor.matmul(pb[:, 0:512], SELPB[:],
                                     VMBP[:, pi_ * N:pi_ * N + 512])
                    nc.tensor.matmul(pb[:, 512:N], SELPB[:],
                                     VMBP[:, pi_ * N + 512:(pi_ + 1) * N])
                    # ACT: relu(t - vb) from PSUM, bf16 out to SBUF + accum
                    ja = jAp.tile([128, N], bf16, tag="jA", name="jA")
                    nc.scalar.activation(out=ja[:], in_=pb[:], func=Act.Relu,
                                         bias=tcol, scale=-1.0,
                                         accum_out=WA[:, u:u + 1])
                    # Pool: mask = (relu out > 0)  <=>  vb < t  (GPSIMD can
                    # neither read PSUM nor accumulate; PE colsums the mask
                    # into the global count row q1)
                    mq = jPp.tile([128, N], bf16, tag="jP", name="jP")
                    nc.gpsimd.tensor_scalar(out=mq[:], in0=ja[:],
                                            scalar1=0.0, scalar2=None,
                                            op0=Alu.is_gt)
                    np_seen[0] += 1
                    if not DBG_NO_Q:
                        pend_mq.append((mq, np_seen[0] == 1))
                        if len(pend_mq) >= 2 or np_seen[0] == NPU:
                            flush_colsums(final=np_seen[0] == NPU)

            # ---- outputs ----
            QC = sg.tile([1, N], f32)
            if DBG_NO_P or DBG_NO_Q:
                nc.vector.memset(QC[:], 0.0)
            else:
                nc.vector.tensor_copy(QC[:], q1[:])
            nc.sync.dma_start(out=rd_o[:], in_=RD[:])
            nc.sync.dma_start(out=wd_o[:], in_=WD[:])
            nc.sync.dma_start(out=wa_o[:], in_=WA[:])
            nc.sync.dma_start(out=qc_o[:], in_=QC[:])
            nc.sync.dma_start(out=tsum_o[:], in_=TSUM[:])

    nc.compile()
    return nc


def _get_program():
    if "nc" not in _CACHE:
        _CACHE["nc"] = _build_program()
    return _CACHE["nc"]


# ---- host-side unit construction (shared by builder consts + decode) ----

def _plan(lab, split=None):
    """Global pairing plan: layouts (labelA, labelB|None) and unit list.
    Side A of a pair must have <= split members, side B <= 128 - split, so
    side B can sit at the fixed partition offset `split` in the shared
    program."""
    import collections
    if split is None:
        split = SPLIT
    cnt = collections.Counter(lab.tolist())
    order = [l for l, _ in sorted(cnt.items(), key=lambda kv: -kv[1])]
    used = set()
    layouts = []          # (labA, labB or None)
    for la in order:
        if la in used:
            continue
        used.add(la)
        lb = None
        for l2 in order:
            if l2 in used:
                continue
            if cnt[la] <= split and cnt[l2] <= 128 - split:
                lb = l2
            elif cnt[l2] <= split and cnt[la] <= 128 - split:
                la, lb = l2, la
            else:
                continue
            used.add(l2)
            break
        layouts.append((la, lb))
    members = {l: np.where(lab == l)[0] for l in cnt}
    units = []            # (layout_idx, anchorA, anchorB or -1)
    for li, (la, lb) in enumerate(layouts):
        ma = members[la]
        mb = members[lb] if lb is not None else np.array([], np.int64)
        npair = min(len(ma), len(mb))
        for i in range(npair):
            units.append((li, int(ma[i]), int(mb[i])))
        big = ma if len(ma) >= len(mb) else mb
        for i in range(npair, len(big)):
            units.append((li, int(big[i]), -1))
    return layouts, members, units


def _configure(lab):
    """Size the per-core program from the actual label distribution; pick
    the block split offset that minimizes the unit count."""
    global UNITS, ALOC, SPLIT, ND
    best = None
    for s in range(32, 97):
        n = len(_plan(lab, s)[2])
        if best is None or n < best[0]:
            best = (n, s)
    u = -(-best[0] // NCORES)
    assert 2 * u <= 128, f"ALOC {2 * u} > 128"
    if u != UNITS or best[1] != SPLIT:
        assert "nc" not in _CACHE, "program already compiled with old config"
        UNITS = u
        ALOC = 2 * u
        SPLIT = best[1]
    nd = UNITS - max(4, UNITS // 3)
    if nd != ND:
        assert "nc" not in _CACHE
        ND = nd


def _core_layouts(lab):
    layouts, members, units = _plan(lab)
    # pad with dummy units (layout -1)
    units = units + [(-1, -1, -1)] * (NCORES * UNITS - len(units))
    per_core = []
    for r in range(NCORES):
        chunk = units[r * UNITS:(r + 1) * UNITS]
        used = []
        for li, _, _ in chunk:
            if li >= 0 and li not in used:
                used.append(li)
        assert len(used) <= NBLK, f"core {r}: {len(used)} layouts"
        # k-permutation: tile c hosts layout used[c]; side A block at row 0,
        # side B block at row SPLIT of the tile.
        perm = np.full(N, -1, np.int64)
        blocked = []
        for c, li in enumerate(used):
            la, lb = layouts[li]
            ma = members[la]
            perm[128 * c:128 * c + len(ma)] = ma
            blocked.append(ma)
            if lb is not None:
                mb = members[lb]
                perm[128 * c + SPLIT:128 * c + SPLIT + len(mb)] = mb
                blocked.append(mb)
        blk = np.concatenate(blocked) if blocked else np.array([], np.int64)
        filler = np.setdiff1d(np.arange(N), blk)
        perm[perm == -1] = filler
        # anchors + select masks; slot u = side A, slot UNITS+u = side B
        anchors = np.zeros(ALOC, np.int64)
        selmask = np.zeros((128, NBLK, 2, UNITS), np.float32)
        for ui, (li, aa, ab) in enumerate(chunk):
            if li < 0:
                continue
            c = used.index(li)
            la, lb = layouts[li]
            ma = members[la]
            anchors[ui] = aa
            anchors[UNITS + ui] = ab if ab >= 0 else aa
            if ab >= 0:
                mb = members[lb]
                selmask[0:len(ma), c, 0, ui] = 1.0
                selmask[int(np.where(ma == aa)[0][0]), c, 0, ui] = 0.0
                selmask[SPLIT:SPLIT + len(mb), c, 1, ui] = 1.0
                selmask[SPLIT + int(np.where(mb == ab)[0][0]),
                        c, 1, ui] = 0.0
            else:
                in_a = bool(np.isin(aa, ma))
                mown = ma if in_a else members[lb]
                off = 0 if in_a else SPLIT
                selmask[off:off + len(mown), c, 0, ui] = 1.0
                selmask[off + int(np.where(mown == aa)[0][0]),
                        c, 0, ui] = 0.0
        per_core.append((anchors, perm, selmask))
    return per_core


def _make_inputs(embeddings, labels):
    e = np.ascontiguousarray(embeddings.reshape(N, D).astype(np.float32))
    lab = labels.reshape(N).astype(np.int64)
    eT = np.ascontiguousarray(e.T)                        # [D, N]
    sq = (e * e).sum(1).astype(np.float32)                # [N] |e|^2
    labf = lab.astype(np.float32)
    selpat = np.zeros((2, 128), np.float32)
    selpat[0, :SPLIT] = 1.0
    selpat[1, SPLIT:] = 1.0
    in_maps = []
    for anchors, perm, selmask in _core_layouts(lab):
        eqm = (labf[anchors][:, None] == labf[perm][None, :])
        in_maps.append({
            "efT2": np.ascontiguousarray(-2.0 * eT[:, perm]).astype(BF16),
            "elocT": np.ascontiguousarray(eT[:, anchors]).astype(BF16),
            "sqfb": sq[perm].reshape(1, N).astype(BF16),
            "sqlb": sq[anchors].reshape(1, ALOC).astype(BF16),
            "eqb": (eqm * np.float32(BIG)).astype(BF16),
            "selm": np.ascontiguousarray(
                selmask.reshape(128, NBLK * 2 * UNITS)).astype(BF16),
            "selpat": selpat.astype(BF16),
        })
    return in_maps


def _decode(res):
    count = 0.0
    total = 0.0
    for r in range(NCORES):
        out = res.results[r]
        count += float(np.asarray(out["rd"]).astype(np.float64).sum())
        count += float(np.asarray(out["qc"]).astype(np.float64).sum())
        total += float(np.asarray(out["wa"]).astype(np.float64).sum())
        wd = np.asarray(out["wd"]).astype(np.float64)
        tsum = np.asarray(out["tsum"]).astype(np.float64).reshape(-1)
        for u in range(ND):
            if _relu_engine(u) == "D":
                total += N * tsum[u] - wd[:, u].sum()
    return total, count


def run_on_device(embeddings: np.ndarray, labels: np.ndarray, **run_kwargs):
    from concourse.bass_utils import run_bass_kernel_spmd
    _configure(np.asarray(labels).reshape(N).astype(np.int64))
    nc = _get_program()
    in_maps = _make_inputs(embeddings, labels)
    res = run_bass_kernel_spmd(nc, in_maps, core_ids=list(range(NCORES)),
                               **run_kwargs)
    total, count = _decode(res)
    return total, count, res


def kernel(embeddings: np.ndarray, labels: np.ndarray):
    embeddings = np.asarray(embeddings)
    labels = np.asarray(labels)
    total, count, _ = run_on_device(embeddings, labels)

    lab = np.asarray(labels).reshape(-1)
    cnt = np.bincount(lab.astype(np.int64), minlength=1)
    per = cnt[lab.astype(np.int64)]
    num_valid = int(((per - 1) * (N - per)).sum())

    nv = np.float32(num_valid)
    ne = np.float32(count)
    tot = np.float32(total)
    if ne > 0:
        loss = np.float32(tot / np.maximum(ne, np.float32(1.0)))
    else:
        loss = np.float32(0.0)
    frac = np.float32(ne / (nv + np.float32(1e-16)))
    return (np.array(loss, np.float32), np.array(nv, np.float32),
            np.array(ne, np.float32), np.array(frac, np.float32))


# revision 47
# speedup vs baseline: 2.4475x; 2.4475x over previous
"""BatchAllTripletLoss on 8 Trainium2 NeuronCores via Bass/Tile.

Math: for anchors i, positives j (same label, j!=i), negatives k (diff label):
  total        = sum_{i,j,k} relu(d_ij - d_ik + margin)
  num_non_easy = #{(i,j,k): d_ik < d_ij + margin}
  loss         = total / num_non_easy ; frac = num_non_easy / num_valid

Strategy. Anchors are grouped into UNITS of up to two anchors from two
different labels (side A at partition 0, side B at partition SPLIT).
Host-built select-masks compact each unit's positive thresholds
t = d_ij + margin onto the 128 partitions; the unit's compare input is
v' = sqrt(d2 + BIG*same_label) for both anchors, broadcast to [128, 640].

Per unit, two reduction passes produce everything via engine accumulators
(accum_out), read back as per-unit columns and summed on host:
  count row sums: R += sum_k (v' < t)        [is_lt, accum]
  relu row sums:  W += sum_k min(v'-t, 0)    [= -relu sums; DVE/Pool]
               or W += sum_k relu(t-v')      [ACT Relu, scale=-1, bias=t]

The v' broadcast is fed two ways to balance engines:
  D-units: DRAM-staged broadcast DMA (partition-stride-0 src) into bf16
    group tiles; compare+relu on DVE (2x bf16 mode) / Pool / ACT.
  P-units: PE broadcast matmul (constant selector weights, staged 2-row
    rhs at partitions 0-1) into PSUM f32; Pool compares, ACT relu-sums.

Distances: one bf16 PE matmul chain per core (dot + rank-1 norm updates);
norms, the BIG same-label mask, and all bf16 casts are host-precomputed.
num_valid is pure label counting (host, exact).
"""

import os
import numpy as np
import ml_dtypes

BF16 = ml_dtypes.bfloat16

# bisection switches (crash isolation); default off
DBG_NO_TSUM = bool(int(os.environ.get("DBG_NO_TSUM", "0")))
DBG_NO_P = bool(int(os.environ.get("DBG_NO_P", "0")))
DBG_ALL_ACT = bool(int(os.environ.get("DBG_ALL_ACT", "0")))
DBG_NO_Q = bool(int(os.environ.get("DBG_NO_Q", "0")))

N = 640
D = 128
NCORES = 8
UNITS = 60                    # units per core (re-derived from labels)
ALOC = 2 * UNITS              # anchor slots per core (2 per unit)
NBLK = 3                      # k-tiles that may hold label blocks
SPLIT = 64                    # partition row where side-B blocks start
MARGIN = 1.9
BIG = 1.0e9
PREFETCH = 2                  # vb group prefetch depth
ND = 40                       # D-units (DMA-fed); rest are P-units (PE-fed)


def _vb_groups():
    """Ramped vb group sizes summing to ND: early compares start after a
    small first transfer instead of a multi-MB one."""
    sizes = []
    for s in (2, 2, 4):
        if sum(sizes) + s <= ND:
            sizes.append(s)
    while sum(sizes) < ND:
        sizes.append(min(8, ND - sum(sizes)))
    return sizes

_CACHE = {}

def _relu_engine(u):
    """Engine for the relu-sum pass of unit u: ACT accum or DVE accum.
    P-units always ACT (they read PSUM, which only ACT/DVE can)."""
    if u >= ND or DBG_ALL_ACT:
        return "A"
    return "D" if u % 5 < 2 else "A"


def _count_accum(u):
    """True: count via DVE accum (1x). False: DVE 2x mask + PE colsum."""
    return u < ND and u % 7 == 0


def _build_program():
    import concourse.bass as bass
    import concourse.bacc as bacc
    import concourse.mybir as mybir
    import concourse.tile as tile
    from concourse.masks import make_identity

    f32 = mybir.dt.float32
    bf16 = mybir.dt.bfloat16
    Alu = mybir.AluOpType
    Act = mybir.ActivationFunctionType

    NP = UNITS - ND           # P-units

    nc = bacc.Bacc("TRN2", target_bir_lowering=False, debug=False,
                   num_devices=NCORES)

    # efT columns pre-scaled by -2 on host; all bf16 host-precast.
    efT2 = nc.declare_dram_parameter("efT2", [D, N], bf16, isOutput=False)
    elocT = nc.declare_dram_parameter("elocT", [D, ALOC], bf16, isOutput=False)
    sqfb = nc.declare_dram_parameter("sqfb", [1, N], bf16, isOutput=False)
    sqlb = nc.declare_dram_parameter("sqlb", [1, ALOC], bf16, isOutput=False)
    eqb = nc.declare_dram_parameter("eqb", [ALOC, N], bf16, isOutput=False)
    selm = nc.declare_dram_parameter("selm", [128, NBLK * 2 * UNITS], bf16,
                                     isOutput=False)
    selpat = nc.declare_dram_parameter("selpat", [2, 128], bf16,
                                       isOutput=False)
    rd_o = nc.declare_dram_parameter("rd", [128, UNITS], f32, isOutput=True)
    wd_o = nc.declare_dram_parameter("wd", [128, UNITS], f32, isOutput=True)
    wa_o = nc.declare_dram_parameter("wa", [128, UNITS], f32, isOutput=True)
    qc_o = nc.declare_dram_parameter("qc", [1, N], f32, isOutput=True)
    tsum_o = nc.declare_dram_parameter("tsum", [1, UNITS], f32, isOutput=True)

    with tile.TileContext(nc) as tc:
        with (
            tc.tile_pool(name="singles", bufs=1) as sg,
            tc.tile_pool(name="vbp", bufs=PREFETCH + 1) as vbp,
            tc.tile_pool(name="jD", bufs=4) as jDp,
            tc.tile_pool(name="jP", bufs=6) as jPp,
            tc.tile_pool(name="jA", bufs=3) as jAp,
            tc.tile_pool(name="dram", bufs=1, space="DRAM") as dram,
            tc.tile_pool(name="ps_a", bufs=1, space="PSUM") as ps_a,
            tc.tile_pool(name="ps_pb", bufs=3, space="PSUM") as ps_pb,
        ):
            # ---- load inputs ----
            EF2 = sg.tile([D, N], bf16)
            nc.sync.dma_start(out=EF2[:], in_=efT2[:])
            EL = sg.tile([D, ALOC], bf16)
            nc.scalar.dma_start(out=EL[:], in_=elocT[:])
            SQFB = sg.tile([1, N], bf16)
            nc.scalar.dma_start(out=SQFB[:], in_=sqfb[:])
            SQLB = sg.tile([1, ALOC], bf16)
            nc.scalar.dma_start(out=SQLB[:], in_=sqlb[:])
            SELPB = sg.tile([2, 128], bf16)
            nc.scalar.dma_start(out=SELPB[:], in_=selpat[:])
            EQBB = sg.tile([ALOC, N], bf16)
            nc.sync.dma_start(out=EQBB[:], in_=eqb[:])
            SELM = sg.tile([128, NBLK * 2 * UNITS], bf16)
            nc.gpsimd.dma_start(out=SELM[:], in_=selm[:])

            ident = sg.tile([128, 128], f32)
            make_identity(nc, ident[:])
            ONESB = sg.tile([1, N], bf16)
            nc.vector.memset(ONESB[:], 1.0)
            ONESC = sg.tile([128, 1], bf16)
            nc.vector.memset(ONESC[:], 1.0)
            ONESCF = sg.tile([128, 1], f32)
            nc.vector.memset(ONESCF[:], 1.0)

            # accumulator result tiles (zeroed: dummy cols stay 0)
            RD = sg.tile([128, UNITS], f32)
            nc.vector.memset(RD[:], 0.0)
            WD = sg.tile([128, UNITS], f32)
            nc.vector.memset(WD[:], 0.0)
            WA = sg.tile([128, UNITS], f32)
            nc.vector.memset(WA[:], 0.0)

            # ---- squared distances fully in PSUM (bf16 matmuls) ----
            # d2[a,k] = (-2 e_a).e_k + |e_k|^2 + |e_a|^2
            dot_ps = ps_a.tile([ALOC, N], f32, tag="a", name="dot")
            for c0, c1 in ((0, 512), (512, N)):
                nc.tensor.matmul(dot_ps[:, c0:c1], EL[:], EF2[:, c0:c1],
                                 start=True, stop=False)
                nc.tensor.matmul(dot_ps[:, c0:c1], ONESB[:, 0:ALOC],
                                 SQFB[:, c0:c1], start=False, stop=False)
                nc.tensor.matmul(dot_ps[:, c0:c1], SQLB[:],
                                 ONESB[:, 0:c1 - c0], start=False, stop=True)

            # PRE2 = max(dot,0); PREM = PRE2 + BIG*eq; VMB = sqrt(PREM) bf16
            PRE2 = sg.tile([ALOC, N], f32)
            nc.vector.tensor_scalar(out=PRE2[:], in0=dot_ps[:], scalar1=0.0,
                                    scalar2=None, op0=Alu.max)
            PREM = sg.tile([ALOC, N], f32)
            nc.vector.scalar_tensor_tensor(out=PREM[:], in0=dot_ps[:],
                                           scalar=0.0, in1=EQBB[:],
                                           op0=Alu.max, op1=Alu.add)
            VMB = sg.tile([ALOC, N], bf16)
            nc.scalar.activation(out=VMB[:], in_=PREM[:], func=Act.Sqrt)
            DIST = sg.tile([ALOC, N], f32)
            nc.scalar.activation(out=DIST[:], in_=PRE2[:], func=Act.Sqrt)

            # ---- P-unit staging: rows (A, B) of each P-unit at parts 0/1
            VMBP = sg.tile([2, NP * N], bf16)
            nc.gpsimd.dma_start(out=VMBP[0:1, :], in_=VMB[ND:UNITS, :])
            nc.gpsimd.dma_start(out=VMBP[1:2, :],
                                in_=VMB[UNITS + ND:2 * UNITS, :])

            # ---- D-unit staging: VMB rows for D-units to DRAM (broadcast
            # DMA requires a DRAM source; SBUF APs reject partition step 0)
            # NOTE: the staging write must be on a DIFFERENT queue than the
            # vb broadcast reads below (sync): same-queue DMAs issue FIFO
            # but complete out of order across the 16 engines, so a reader
            # on the same queue can see stale DRAM.
            vmd = dram.tile([2, ND, N], bf16)
            nc.scalar.dma_start(out=vmd[0:1, :, :], in_=VMB[0:ND, :])
            nc.scalar.dma_start(out=vmd[1:2, :, :],
                                in_=VMB[UNITS:UNITS + ND, :])

            # ---- vb group DMA for D-units (DRAM->SBUF broadcast) ----
            # Triggers go on the otherwise-idle SP (sync) queue; ramped
            # group sizes so the first compares start after a small
            # transfer instead of a multi-MB one.
            VB_GROUPS = _vb_groups()
            gstart = []
            s = 0
            for gsz in VB_GROUPS:
                gstart.append(s)
                s += gsz
            assert s == ND, (s, ND)
            MAXG = max(VB_GROUPS)
            vb_tiles = {}

            def issue_vb(g):
                if g >= len(VB_GROUPS):
                    return
                u0 = gstart[g]
                nu = VB_GROUPS[g]
                vb8 = vbp.tile([128, MAXG, N], bf16, tag="vb", name="vb")
                pitch = MAXG * N
                for half, (p0, np_) in enumerate(((0, SPLIT),
                                                  (SPLIT, 128 - SPLIT))):
                    dst = bass.AP(tensor=vb8.tensor,
                                  offset=vb8.offset + p0 * pitch,
                                  ap=[[pitch, np_], [N, nu], [1, N]])
                    src = bass.AP(tensor=vmd.tensor,
                                  offset=vmd.offset + (half * ND + u0) * N,
                                  ap=[[0, np_], [N, nu], [1, N]])
                    nc.gpsimd.dma_start(out=dst, in_=src)
                vb_tiles[g] = vb8

            u2g = {}
            for g, (u0, gsz) in enumerate(zip(gstart, VB_GROUPS)):
                for i in range(gsz):
                    u2g[u0 + i] = g

            for g in range(PREFETCH):
                issue_vb(g)

            # ---- thresholds: TSEL[p,u] from transposed DIST + masks ----
            TSEL = sg.tile([128, UNITS], f32)
            first = True
            for c in range(NBLK):
                tr_ps = ps_a.tile([128, ALOC], f32, tag="a", name=f"tr{c}")
                nc.tensor.transpose(tr_ps[:], DIST[:, c * 128:(c + 1) * 128],
                                    ident[0:ALOC, 0:ALOC])
                for side in range(2):
                    sel = SELM[:, (2 * c + side) * UNITS:
                               (2 * c + side + 1) * UNITS]
                    if first:
                        nc.vector.scalar_tensor_tensor(
                            out=TSEL[:],
                            in0=tr_ps[:, side * UNITS:(side + 1) * UNITS],
                            scalar=MARGIN, in1=sel,
                            op0=Alu.add, op1=Alu.mult)
                        first = False
                    else:
                        t = sg.tile([128, UNITS], f32, tag="tstmp",
                                    name="tstmp")
                        nc.vector.scalar_tensor_tensor(
                            out=t[:],
                            in0=tr_ps[:, side * UNITS:(side + 1) * UNITS],
                            scalar=MARGIN, in1=sel,
                            op0=Alu.add, op1=Alu.mult)
                        nc.vector.tensor_add(TSEL[:], TSEL[:], t[:])

            # ---- main loop: riffle P and D units (P first: their staging
            # DMA is ready before the D-units' DRAM round-trip) ----
            order = []
            di, pi = 0, ND
            while di < ND or pi < UNITS:
                take_p = (pi - ND) * ND <= di * (UNITS - ND)
                if pi < UNITS and (take_p or di >= ND):
                    order.append(pi); pi += 1
                else:
                    order.append(di); di += 1

            # per-unit threshold column sums (exact, device-side): the DVE
            # relu pass accumulates sum_k min(vb, t) and the host recovers
            # sum relu(t-vb) = 640*tsum - that, so tsum must match the
            # device's own t values bit-for-bit (no host sqrt recompute).
            TSUM = sg.tile([1, UNITS], f32)
            if not DBG_NO_TSUM:
                ts_ps = ps_a.tile([1, UNITS], f32, tag="a", name="tsum")
                nc.tensor.matmul(ts_ps[:], ONESCF[:], TSEL[:])
                nc.vector.tensor_copy(TSUM[:], ts_ps[:])
            else:
                nc.vector.memset(TSUM[:], 0.0)

            # global count accumulator q1 (reuses the startup psum
            # slot; its last prior tile, ts_ps, is consumed by the copy).
            # Counts flow in as DVE 2x masks reduced by PE colsums; a few
            # D-units use the (1x) DVE accumulator instead to balance.
            q1 = ps_a.tile([1, N], f32, tag="a", name="q1")
            pend_mq = []          # masks awaiting a PE colsum (LDW batching)
            n_masks = sum(0 if _count_accum(u) else 1 for u in range(UNITS))
            seen = [0]

            def flush_colsums():
                for mq in pend_mq:
                    seen[0] += 1
                    first = seen[0] == 1
                    last = seen[0] == n_masks or mq is pend_mq[-1]
                    nc.tensor.matmul(q1[:, 0:512], ONESC[:], mq[:, 0:512],
                                     start=first, stop=last,
                                     skip_group_check=True)
                    nc.tensor.matmul(q1[:, 512:N], ONESC[:], mq[:, 512:N],
                                     start=first, stop=last,
                                     skip_group_check=True)
                pend_mq.clear()

            def add_mask(mq):
                pend_mq.append(mq)
                if len(pend_mq) >= 2 or seen[0] + len(pend_mq) == n_masks:
                    flush_colsums()

            for u in order:
                if u >= ND and DBG_NO_P:
                    continue
                tcol = TSEL[:, u:u + 1]
                if u < ND:
                    g = u2g[u]
                    if u == gstart[g]:
                        issue_vb(g + PREFETCH)
                    vb = vb_tiles[g][:, u - gstart[g], :]
                    if _count_accum(u):
                        md = jDp.tile([128, N], bf16, tag="jD", name="jD")
                        nc.vector.tensor_scalar(out=md[:], in0=vb,
                                                scalar1=tcol, scalar2=0.0,
                                                op0=Alu.is_lt, op1=Alu.add,
                                                accum_out=RD[:, u:u + 1])
                    else:
                        mq = jPp.tile([128, N], bf16, tag="jP", name="jP")
                        nc.vector.tensor_scalar(out=mq[:], in0=vb,
                                                scalar1=tcol, scalar2=None,
                                                op0=Alu.is_lt)
                        add_mask(mq)
                    if _relu_engine(u) == "D":
                        # accum (op1) is the REDUCE op; op0=min gives
                        # sum_k min(vb, t) = 640*t - sum_k relu(t - vb)
                        md2 = jDp.tile([128, N], bf16, tag="jD", name="jD2")
                        nc.vector.tensor_scalar(out=md2[:], in0=vb,
                                                scalar1=tcol, scalar2=0.0,
                                                op0=Alu.min, op1=Alu.add,
                                                accum_out=WD[:, u:u + 1])
                    else:
                        ja2 = jAp.tile([128, N], bf16, tag="jA", name="jA2")
                        nc.scalar.activation(out=ja2[:], in_=vb,
                                             func=Act.Relu, bias=tcol,
                                             scale=-1.0,
                                             accum_out=WA[:, u:u + 1])
                else:
                    pi_ = u - ND
                    pb = ps_pb.tile([128, N], f32, tag="pb", name="pb")
                    nc.tensor.matmul(pb[:, 0:512], SELPB[:],
                                     VMBP[:, pi_ * N:pi_ * N + 512])
                    nc.tensor.matmul(pb[:, 512:N], SELPB[:],
                                     VMBP[:, pi_ * N + 512:(pi_ + 1) * N])
                    # ACT: relu(t - vb) from PSUM, bf16 out to SBUF + accum
                    ja = jAp.tile([128, N], bf16, tag="jA", name="jA")
                    nc.scalar.activation(out=ja[:], in_=pb[:], func=Act.Relu,
                                         bias=tcol, scale=-1.0,
                                         accum_out=WA[:, u:u + 1])
                    # DVE 2x: count mask = (relu out > 0) <=> vb < t
                    mq = jPp.tile([128, N], bf16, tag="jP", name="jP")
                    nc.vector.tensor_scalar(out=mq[:], in0=ja[:],
                                            scalar1=0.0, scalar2=None,
                                            op0=Alu.is_gt)
                    add_mask(mq)

            # ---- outputs ----
            QC = sg.tile([1, N], f32)
            nc.vector.tensor_copy(QC[:], q1[:])
            nc.sync.dma_start(out=rd_o[:], in_=RD[:])
            nc.sync.dma_start(out=wd_o[:], in_=WD[:])
            nc.sync.dma_start(out=wa_o[:], in_=WA[:])
            nc.sync.dma_start(out=qc_o[:], in_=QC[:])
            nc.sync.dma_start(out=tsum_o[:], in_=TSUM[:])

    nc.compile()
    return nc


def _get_program():
    if "nc" not in _CACHE:
        _CACHE["nc"] = _build_program()
    return _CACHE["nc"]


# ---- host-side unit construction (shared by builder consts + decode) ----

def _plan(lab, split=None):
    """Global pairing plan: layouts (labelA, labelB|None) and unit list.
    Side A of a pair must have <= split members, side B <= 128 - split, so
    side B can sit at the fixed partition offset `split` in the shared
    program."""
    import collections
    if split is None:
        split = SPLIT
    cnt = collections.Counter(lab.tolist())
    order = [l for l, _ in sorted(cnt.items(), key=lambda kv: -kv[1])]
    used = set()
    layouts = []          # (labA, labB or None)
    for la in order:
        if la in used:
            continue
        used.add(la)
        lb = None
        for l2 in order:
            if l2 in used:
                continue
            if cnt[la] <= split and cnt[l2] <= 128 - split:
                lb = l2
            elif cnt[l2] <= split and cnt[la] <= 128 - split:
                la, lb = l2, la
            else:
                continue
            used.add(l2)
            break
        layouts.append((la, lb))
    members = {l: np.where(lab == l)[0] for l in cnt}
    units = []            # (layout_idx, anchorA, anchorB or -1)
    for li, (la, lb) in enumerate(layouts):
        ma = members[la]
        mb = members[lb] if lb is not None else np.array([], np.int64)
        npair = min(len(ma), len(mb))
        for i in range(npair):
            units.append((li, int(ma[i]), int(mb[i])))
        big = ma if len(ma) >= len(mb) else mb
        for i in range(npair, len(big)):
            units.append((li, int(big[i]), -1))
    return layouts, members, units


def _configure(lab):
    """Size the per-core program from the actual label distribution; pick
    the block split offset that minimizes the unit count."""
    global UNITS, ALOC, SPLIT, ND
    best = None
    for s in range(32, 97):
        n = len(_plan(lab, s)[2])
        if best is None or n < best[0]:
            best = (n, s)
    u = -(-best[0] // NCORES)
    assert 2 * u <= 128, f"ALOC {2 * u} > 128"
    if u != UNITS or best[1] != SPLIT:
        assert "nc" not in _CACHE, "program already compiled with old config"
        UNITS = u
        ALOC = 2 * u
        SPLIT = best[1]
    nd = UNITS - max(4, UNITS // 3)
    if nd != ND:
        assert "nc" not in _CACHE
        ND = nd


def _core_layouts(lab):
    layouts, members, units = _plan(lab)
    # pad with dummy units (layout -1)
    units = units + [(-1, -1, -1)] * (NCORES * UNITS - len(units))
    per_core = []
    for r in range(NCORES):
        chunk = units[r * UNITS:(r + 1) * UNITS]
        used = []
        for li, _, _ in chunk:
            if li >= 0 and li not in used:
                used.append(li)
        assert len(used) <= NBLK, f"core {r}: {len(used)} layouts"
        # k-permutation: tile c hosts layout used[c]; side A block at row 0,
        # side B block at row SPLIT of the tile.
        perm = np.full(N, -1, np.int64)
        blocked = []
        for c, li in enumerate(used):
            la, lb = layouts[li]
            ma = members[la]
            perm[128 * c:128 * c + len(ma)] = ma
            blocked.append(ma)
            if lb is not None:
                mb = members[lb]
                perm[128 * c + SPLIT:128 * c + SPLIT + len(mb)] = mb
                blocked.append(mb)
        blk = np.concatenate(blocked) if blocked else np.array([], np.int64)
        filler = np.setdiff1d(np.arange(N), blk)
        perm[perm == -1] = filler
        # anchors + select masks; slot u = side A, slot UNITS+u = side B
        anchors = np.zeros(ALOC, np.int64)
        selmask = np.zeros((128, NBLK, 2, UNITS), np.float32)
        for ui, (li, aa, ab) in enumerate(chunk):
            if li < 0:
                continue
            c = used.index(li)
            la, lb = layouts[li]
            ma = members[la]
            anchors[ui] = aa
            anchors[UNITS + ui] = ab if ab >= 0 else aa
            if ab >= 0:
                mb = members[lb]
                selmask[0:len(ma), c, 0, ui] = 1.0
                selmask[int(np.where(ma == aa)[0][0]), c, 0, ui] = 0.0
                selmask[SPLIT:SPLIT + len(mb), c, 1, ui] = 1.0
                selmask[SPLIT + int(np.where(mb == ab)[0][0]),
                        c, 1, ui] = 0.0
            else:
                in_a = bool(np.isin(aa, ma))
                mown = ma if in_a else members[lb]
                off = 0 if in_a else SPLIT
                selmask[off:off + len(mown), c, 0, ui] = 1.0
                selmask[off + int(np.where(mown == aa)[0][0]),
                        c, 0, ui] = 0.0
        per_core.append((anchors, perm, selmask))
    return per_core


def _make_inputs(embeddings, labels):
    e = np.ascontiguousarray(embeddings.reshape(N, D).astype(np.float32))
    lab = labels.reshape(N).astype(np.int64)
    eT = np.ascontiguousarray(e.T)                        # [D, N]
    sq = (e * e).sum(1).astype(np.float32)                # [N] |e|^2
    labf = lab.astype(np.float32)
    selpat = np.zeros((2, 128), np.float32)
    selpat[0, :SPLIT] = 1.0
    selpat[1, SPLIT:] = 1.0
    in_maps = []
    for anchors, perm, selmask in _core_layouts(lab):
        eqm = (labf[anchors][:, None] == labf[perm][None, :])
        in_maps.append({
            "efT2": np.ascontiguousarray(-2.0 * eT[:, perm]).astype(BF16),
            "elocT": np.ascontiguousarray(eT[:, anchors]).astype(BF16),
            "sqfb": sq[perm].reshape(1, N).astype(BF16),
            "sqlb": sq[anchors].reshape(1, ALOC).astype(BF16),
            "eqb": (eqm * np.float32(BIG)).astype(BF16),
            "selm": np.ascontiguousarray(
                selmask.reshape(128, NBLK * 2 * UNITS)).astype(BF16),
            "selpat": selpat.astype(BF16),
        })
    return in_maps


def _decode(res):
    count = 0.0
    total = 0.0
    for r in range(NCORES):
        out = res.results[r]
        count += float(np.asarray(out["rd"]).astype(np.float64).sum())
        count += float(np.asarray(out["qc"]).astype(np.float64).sum())
        total += float(np.asarray(out["wa"]).astype(np.float64).sum())
        wd = np.asarray(out["wd"]).astype(np.float64)
        tsum = np.asarray(out["tsum"]).astype(np.float64).reshape(-1)
        for u in range(ND):
            if _relu_engine(u) == "D":
                total += N * tsum[u] - wd[:, u].sum()
    return total, count


def run_on_device(embeddings: np.ndarray, labels: np.ndarray, **run_kwargs):
    from concourse.bass_utils import run_bass_kernel_spmd
    _configure(np.asarray(labels).reshape(N).astype(np.int64))
    nc = _get_program()
    in_maps = _make_inputs(embeddings, labels)
    res = run_bass_kernel_spmd(nc, in_maps, core_ids=list(range(NCORES)),
                               **run_kwargs)
    total, count = _decode(res)
    return total, count, res


def kernel(embeddings: np.ndarray, labels: np.ndarray):
    embeddings = np.asarray(embeddings)
    labels = np.asarray(labels)
    total, count, _ = run_on_device(embeddings, labels)

    lab = np.asarray(labels).reshape(-1)
    cnt = np.bincount(lab.astype(np.int64), minlength=1)
    per = cnt[lab.astype(np.int64)]
    num_valid = int(((per - 1) * (N - per)).sum())

    nv = np.float32(num_valid)
    ne = np.float32(count)
    tot = np.float32(total)
    if ne > 0:
        loss = np.float32(tot / np.maximum(ne, np.float32(1.0)))
    else:
        loss = np.float32(0.0)
    frac = np.float32(ne / (nv + np.float32(1e-16)))
    return (np.array(loss, np.float32), np.array(nv, np.float32),
            np.array(ne, np.float32), np.array(frac, np.float32))
